# revision 29
# baseline (speedup 1.0000x reference)
"""Trainium2 Bass kernel for nn_Attention_9320079032376 (v2, chunk-pipelined).

Full attention block: RMSNorm -> QKV proj -> interleaved RoPE -> GQA causal
attention (32 q heads / 8 kv heads, hd=64) -> out proj.  B=2, S=2048, D=2048.

Sharding: 8 cores = 2 batches x 4 kv-head-pairs.  Core c handles batch c//4
and kv heads {2j, 2j+1} (j = c%4) plus their 8 GQA q-heads.  Host pre-casts
to bf16, pre-transposes x, and pre-permutes weight columns (see
prep_core_inputs).

v2 restructure vs v1: the whole computation is pipelined over 4 token chunks
of 512.  Per chunk: RMSNorm stats (squares on GPSIMD, partition-reduce on PE,
sqrt on ACT, fast-reciprocal on DVE) -> QKV projections + RoPE -> causal
attention in transposed orientation (scores^T on PE with K=64 row tiling,
exp on ACT with fused 1/8 scale, no max subtraction) -> per-chunk og
AllGather into a Shared-scratchpad DRAM tensor (overlaps later chunks'
compute) -> per-chunk output projection.  Softmax denominators come from a
ones column in the og matmul (M=65) and are inverted with the approximate
1-instruction DVE reciprocal.
"""
import sys
sys.path.insert(0, "/opt/trn_rl_repo")

import contextlib
import numpy as np
import ml_dtypes

import concourse.bass as bass
import concourse.mybir as mybir
import concourse.tile as tile
from concourse import bacc
from concourse.bass import ts, ds
from concourse.masks import make_identity

BF16 = ml_dtypes.bfloat16
bf16 = mybir.dt.bfloat16
f32 = mybir.dt.float32
fp8 = mybir.dt.float8e4
FP8 = ml_dtypes.float8_e4m3
AF = mybir.ActivationFunctionType
ALU = mybir.AluOpType

B, S, D = 2, 2048, 2048
HEADS, KV, HD = 32, 8, 64
EPS = 1.1920929e-07
THETA = 10000.0
NCORE = 8

PERM64 = np.concatenate([np.arange(0, 64, 2), np.arange(1, 64, 2)])


# ---------------------------------------------------------------- builder
def build_nc(Sx=S, Dx=D, groups=4, num_devices=8, debug=False, reps=1):
    """One SPMD program; per-core behavior differs only via input data."""
    TC = Sx // 512          # token chunks of 512
    DT = Dx // 128          # contraction dim tiles
    NT = Sx // 128          # token tiles of 128
    RG = ([[0, 1, 2, 3], [4, 5, 6, 7]] if groups == 4 else [[0]])
    dbg_kw = {"kind": "ExternalOutput"} if debug else {}

    nc = bacc.Bacc("TRN2", target_bir_lowering=False, debug=False,
                   num_devices=num_devices)
    xbT = nc.dram_tensor("xbT", [Dx, Sx], bf16, kind="ExternalInput")
    wq = nc.dram_tensor("wq", [Dx, 512], bf16, kind="ExternalInput")
    wk = nc.dram_tensor("wk", [Dx, 128], bf16, kind="ExternalInput")
    wv = nc.dram_tensor("wv", [Dx, 128], bf16, kind="ExternalInput")
    wo = nc.dram_tensor("wo", [Dx, 512], bf16, kind="ExternalInput")
    c128 = nc.dram_tensor("c128", [128, Sx], f32, kind="ExternalInput")
    s128 = nc.dram_tensor("s128", [128, Sx], f32, kind="ExternalInput")
    tri = nc.dram_tensor("tri", [128, 128], bf16, kind="ExternalInput")
    perm = nc.dram_tensor("perm", [128, 128], bf16, kind="ExternalInput")
    outT = nc.dram_tensor("outT", [512, Sx], f32, kind="ExternalOutput")
    og_c = [nc.dram_tensor(f"og{t}", [512, 512], bf16) for t in range(TC)]
    ag_c = [nc.dram_tensor(f"ag{t}", [Dx, 512], bf16) for t in range(TC)]
    dbg = {}
    if debug:
        dbg["r"] = nc.dram_tensor("dbg_r", [128, Sx], bf16, kind="ExternalOutput")
        dbg["k"] = nc.dram_tensor("dbg_k", [128, Sx], bf16, kind="ExternalOutput")
        dbg["q"] = nc.dram_tensor("dbg_q", [128, Sx], bf16, kind="ExternalOutput")
        dbg["v"] = nc.dram_tensor("dbg_v", [128, NT * 130], bf16,
                                  kind="ExternalOutput")
        dbg["og"] = [nc.dram_tensor(f"dbg_og{t}", [512, 512], bf16,
                                    kind="ExternalOutput") for t in range(TC)]
        dbg["ag"] = [nc.dram_tensor(f"dbg_ag{t}", [Dx, 512], bf16,
                                    kind="ExternalOutput") for t in range(TC)]

    with tile.TileContext(nc) as tc, contextlib.ExitStack() as ctx:
        const = ctx.enter_context(tc.tile_pool(name="const", bufs=1))
        wpool = ctx.enter_context(tc.tile_pool(name="wpool", bufs=1))
        work = ctx.enter_context(tc.tile_pool(name="work", bufs=1))
        kvp = ctx.enter_context(tc.tile_pool(name="kvp", bufs=1))
        xcp = ctx.enter_context(tc.tile_pool(name="xcp", bufs=2))
        qp = ctx.enter_context(tc.tile_pool(name="qp", bufs=2))
        sqp = ctx.enter_context(tc.tile_pool(name="sqp", bufs=3))
        tmp = ctx.enter_context(tc.tile_pool(name="tmp", bufs=3))
        attp = ctx.enter_context(tc.tile_pool(name="attp", bufs=6))
        ogo = ctx.enter_context(tc.tile_pool(name="ogo", bufs=4))
        nrm = ctx.enter_context(tc.tile_pool(name="nrm", bufs=2))
        ogsb = ctx.enter_context(tc.tile_pool(name="ogsb", bufs=2))
        osb = ctx.enter_context(tc.tile_pool(name="osb", bufs=3))
        sp_ps = ctx.enter_context(tc.tile_pool(name="sp_ps", bufs=2, space="PSUM"))
        og_ps = ctx.enter_context(tc.tile_pool(name="og_ps", bufs=2, space="PSUM"))
        acc_ps = ctx.enter_context(tc.tile_pool(name="acc_ps", bufs=2, space="PSUM"))

        identf = const.tile([128, 128], f32)
        make_identity(nc, identf)
        trit = const.tile([128, 128], bf16)
        nc.gpsimd.dma_start(out=trit[:], in_=tri[:])
        permt = const.tile([128, 128], bf16)
        nc.gpsimd.dma_start(out=permt[:], in_=perm[:])
        onesb = const.tile([128, 1], bf16)
        nc.vector.memset(onesb[:], 1.0)
        ones_f = const.tile([1, 128], f32)
        nc.vector.memset(ones_f[:], 1.0)
        epsb = const.tile([1, 1], f32)
        nc.vector.memset(epsb[:], float(EPS))

        # persistent sbuf tensors
        wq_sb = wpool.tile([128, DT, 512], bf16)
        nc.gpsimd.dma_start(out=wq_sb[:], in_=wq.rearrange("(dt p) c -> p dt c", p=128))
        wk_sb = wpool.tile([128, DT, 128], bf16)
        nc.gpsimd.dma_start(out=wk_sb[:], in_=wk.rearrange("(dt p) c -> p dt c", p=128))
        wv_sb = wpool.tile([128, DT, 128], bf16)
        nc.gpsimd.dma_start(out=wv_sb[:], in_=wv.rearrange("(dt p) c -> p dt c", p=128))
        wo_sb = wpool.tile([128, DT, 512], bf16)
        nc.gpsimd.dma_start(out=wo_sb[:], in_=wo.rearrange("(dt p) c -> p dt c", p=128))

        crt = work.tile([128, Sx], f32, tag="crt")   # cos * r
        srt = work.tile([128, Sx], f32, tag="srt")   # sin(+-) * r
        rb_sb = work.tile([128, Sx], bf16, tag="rb")  # r broadcast (for v)

        kT = kvp.tile([128, Sx], bf16, tag="kT")
        # v: [128 tok, tile, 130]: [vA(64) onesA vB(64) onesB]
        v3d = kvp.tile([128, NT, 130], bf16, tag="v3d")
        nc.vector.memset(v3d[:, :, 64:65], 1.0)
        nc.vector.memset(v3d[:, :, 129:130], 1.0)

        xbTr = xbT.rearrange("(dt p) t -> p dt t", p=128)

        def out_proj(tcc):
            """Output projection for chunk tcc (consumes ag_c[tcc])."""
            src = ag_c[tcc] if groups > 1 else og_c[tcc]
            gdt = DT if groups > 1 else 4
            og_sb = ogsb.tile([128, gdt, 512], bf16, tag="ogsb")
            nc.sync.dma_start(out=og_sb[:],
                              in_=src.rearrange("(dt p) t -> p dt t", p=128))
            for oc in range(4):
                ot = acc_ps.tile([128, 512], f32, tag="acc")
                for dt in range(gdt):
                    nc.tensor.matmul(ot[:], wo_sb[:, dt, ts(oc, 128)],
                                     og_sb[:, dt, :],
                                     start=(dt == 0), stop=(dt == gdt - 1))
                ob = osb.tile([128, 512], f32, tag="ob")
                nc.vector.tensor_copy(ob[:], ot[:])
                nc.sync.dma_start(out=outT[ts(oc, 128), ts(tcc, 512)],
                                  in_=ob[:])

        for rep in range(reps):
         nc.gpsimd.dma_start(out=crt[:], in_=c128[:])
         nc.gpsimd.dma_start(out=srt[:], in_=s128[:])
         for tcc in range(TC):
            sl = ts(tcc, 512)
            # ---------------- x chunk load + RMSNorm stats ----------------
            xc = xcp.tile([128, DT, 512], bf16, tag="xc")
            nc.sync.dma_start(out=xc[:], in_=xbTr[:, :, sl])
            st = acc_ps.tile([1, 512], f32, tag="acc")
            for dt in range(DT):
                sq = sqp.tile([128, 512], bf16, tag="sq")
                nc.vector.tensor_mul(sq[:], xc[:, dt, :], xc[:, dt, :])
                nc.tensor.matmul(st[:], onesb[:], sq[:],
                                 start=(dt == 0), stop=(dt == DT - 1))
            rr = tmp.tile([1, 512], f32, tag="rr")
            nc.scalar.activation(rr[:], st[:], AF.Sqrt,
                                 bias=epsb[:], scale=float(1.0 / Dx))
            rw = tmp.tile([1, 512], f32, tag="rw")
            nc.vector.reciprocal_approx_fast(out=rw[:], in_=rr[:])
            rbp = acc_ps.tile([128, 512], f32, tag="acc")
            nc.tensor.matmul(rbp[:], ones_f[:], rw[:], start=True, stop=True)
            nc.vector.tensor_mul(crt[:, sl], crt[:, sl], rbp[:])
            nc.vector.tensor_mul(srt[:, sl], srt[:, sl], rbp[:])
            nc.vector.tensor_copy(rb_sb[:, sl], rbp[:])

            # ---------------- projections + rope + v ----------------------
            qc = [qp.tile([128, 512], bf16, tag=f"q{i}", name=f"q{i}_{tcc}")
                  for i in range(4)]
            for pk in (4, 5, 0, 1, 2, 3):   # k,v first; then q packs
                pj = acc_ps.tile([128, 512], f32, tag="acc")
                for dt in range(DT):
                    if pk < 4:
                        lhs = wq_sb[:, dt, ts(pk, 128)]
                    elif pk == 4:
                        lhs = wk_sb[:, dt, :]
                    else:
                        lhs = wv_sb[:, dt, :]
                    nc.tensor.matmul(pj[:], lhs, xc[:, dt, :],
                                     start=(dt == 0), stop=(dt == DT - 1))
                if pk < 5:
                    tmpc = tmp.tile([128, 512], bf16, tag="tmpc")
                    tmps = tmp.tile([128, 512], bf16, tag="tmps")
                    nc.vector.tensor_mul(tmpc[:], pj[:], crt[:, sl])
                    nc.vector.tensor_mul(tmps[:], pj[:], srt[:, sl])
                    swp = acc_ps.tile([128, 512], f32, tag="acc")
                    nc.tensor.matmul(swp[:], permt[:], tmps[:],
                                     start=True, stop=True)
                    dest = qc[pk][:] if pk < 4 else kT[:, sl]
                    nc.vector.tensor_add(dest, tmpc[:], swp[:])
                else:
                    vsc = tmp.tile([128, 512], f32, tag="vsc")
                    nc.vector.tensor_mul(vsc[:], pj[:], rb_sb[:, sl])
                    for stt in range(4):
                        tt = 4 * tcc + stt
                        vp = acc_ps.tile([128, 128], f32, tag="acc")
                        nc.tensor.transpose(vp[:], vsc[:, ts(stt, 128)],
                                            identf[:])
                        nc.vector.tensor_copy(v3d[:, tt, 0:64], vp[:, 0:64])
                        nc.vector.tensor_copy(v3d[:, tt, 65:129], vp[:, 64:128])

            # ---- out-projection for the PREVIOUS chunk (its AllGather has
            # had a full attention chunk to complete; placing it here keeps
            # the acc psum pool's in-order slot grants from stalling the
            # next chunk's stats/proj behind the collective)
            if tcc >= 1:
                out_proj(tcc - 1)

            # ---------------- attention for chunk tcc ---------------------
            nkt = 4 * (tcc + 1)
            for pk in range(4):
                og_a = og_ps.tile([128, 512], f32, tag="og")
                og_b = og_ps.tile([128, 512], f32, tag="og")
                for kt in range(nkt):
                    kr = kt - 4 * tcc     # >=0 on diagonal tiles
                    sp = sp_ps.tile([128, 1024], f32, tag="s")
                    pT = attp.tile([128, 1024], bf16, tag="pT")
                    if kr < 0:
                        nc.tensor.matmul(sp[:, 0:512],
                                         kT[0:64, ts(kt, 128)],
                                         qc[pk][0:64, :],
                                         start=True, stop=True,
                                         tile_position=(0, 0))
                        nc.tensor.matmul(sp[:, 512:1024],
                                         kT[64:128, ts(kt, 128)],
                                         qc[pk][64:128, :],
                                         start=True, stop=True,
                                         tile_position=(64, 0))
                        nc.scalar.activation(pT[:], sp[:], AF.Exp, scale=0.125)
                        nc.tensor.matmul(og_a[0:65, :], v3d[:, kt, 0:65],
                                         pT[:, 0:512],
                                         start=(kt == 0), stop=(kt == nkt - 1))
                        nc.tensor.matmul(og_b[0:65, :], v3d[:, kt, 65:130],
                                         pT[:, 512:1024],
                                         start=(kt == 0), stop=(kt == nkt - 1))
                    else:
                        c0 = 128 * kr     # first valid q col in chunk
                        w = 512 - c0
                        nc.tensor.matmul(sp[:, ds(c0, w)],
                                         kT[0:64, ts(kt, 128)],
                                         qc[pk][0:64, ds(c0, w)],
                                         start=True, stop=True,
                                         tile_position=(0, 0))
                        nc.tensor.matmul(sp[:, ds(512 + c0, w)],
                                         kT[64:128, ts(kt, 128)],
                                         qc[pk][64:128, ds(c0, w)],
                                         start=True, stop=True,
                                         tile_position=(64, 0))
                        dg = attp.tile([128, 256], bf16, tag="dg")
                        for h in range(2):
                            base = 512 * h
                            if w > 128:
                                nc.scalar.activation(
                                    pT[:, ds(base + c0 + 128, w - 128)],
                                    sp[:, ds(base + c0 + 128, w - 128)],
                                    AF.Exp, scale=0.125)
                            nc.scalar.activation(dg[:, ts(h, 128)],
                                                 sp[:, ds(base + c0, 128)],
                                                 AF.Exp, scale=0.125)
                            nc.vector.tensor_mul(pT[:, ds(base + c0, 128)],
                                                 dg[:, ts(h, 128)], trit[:])
                        nc.tensor.matmul(og_a[0:65, ds(c0, w)],
                                         v3d[:, kt, 0:65], pT[:, ds(c0, w)],
                                         start=(kt == 0), stop=(kt == nkt - 1))
                        nc.tensor.matmul(og_b[0:65, ds(c0, w)],
                                         v3d[:, kt, 65:130],
                                         pT[:, ds(512 + c0, w)],
                                         start=(kt == 0), stop=(kt == nkt - 1))
                og_out = ogo.tile([128, 512], bf16, tag="ogout")
                for h, ogp in ((0, og_a), (1, og_b)):
                    dn = nrm.tile([1, 512], f32, tag="dn")
                    nc.vector.tensor_copy(dn[:], ogp[64:65, :])
                    rl = nrm.tile([1, 512], f32, tag="rl")
                    nc.vector.reciprocal_approx_fast(out=rl[:], in_=dn[:])
                    bcp = acc_ps.tile([64, 512], f32, tag="acc")
                    nc.tensor.matmul(bcp[:], ones_f[0:1, 0:64], rl[:],
                                     start=True, stop=True)
                    bc = nrm.tile([64, 512], f32, tag="bc")
                    nc.vector.tensor_copy(bc[:], bcp[:])
                    nc.vector.tensor_mul(og_out[ds(64 * h, 64), :],
                                         ogp[0:64, :], bc[:])
                nc.sync.dma_start(out=og_c[tcc][ts(pk, 128), :], in_=og_out[:])

            # ---------------- og AllGather for this chunk -----------------
            if groups > 1:
                nc.gpsimd.collective_compute(
                    "AllGather", ALU.bypass, replica_groups=RG,
                    ins=[og_c[tcc][:]], outs=[ag_c[tcc][:]])
            if debug:
                nc.sync.dma_start(out=dbg["q"][:, sl], in_=qc[0][:])
                nc.sync.dma_start(out=dbg["og"][tcc][:], in_=og_c[tcc][:])
                if groups > 1:
                    nc.sync.dma_start(out=dbg["ag"][tcc][:], in_=ag_c[tcc][:])
         out_proj(TC - 1)
        if debug:
            nc.sync.dma_start(out=dbg["r"][:], in_=rb_sb[:])
            nc.sync.dma_start(out=dbg["k"][:], in_=kT[:])
            nc.sync.dma_start(out=dbg["v"][:],
                              in_=v3d[:].rearrange("p n c -> p (n c)"))
    nc.compile()
    return nc


# ---------------------------------------------------------------- host prep
def _rope_tables(Sx):
    f = np.arange(32)
    invf = THETA ** (-2.0 * f / 64.0)
    t = np.arange(Sx, dtype=np.float64)
    ang = t[None, :] * invf[:, None]
    c = np.tile(np.cos(ang), (4, 1)).astype(np.float32)
    sgn = np.concatenate([np.ones(32), -np.ones(32)] * 2)[:, None]
    s = (np.tile(np.sin(ang), (4, 1)) * sgn).astype(np.float32)
    return c, s


def _tri_mask():
    p = np.arange(128)
    return (p[None, :] >= p[:, None]).astype(BF16)


def _perm128():
    m = np.arange(128)
    sw = np.where((m % 64) < 32, m + 32, m - 32)
    P = np.zeros((128, 128), np.float32)
    P[sw, m] = 1.0     # P[k, m] = 1 iff k == swap(m)
    return P.astype(BF16)


def _ag_head_of_row(r):
    return 2 * (r // 512) + ((r % 128) // 64) + 8 * ((r % 512) // 128)


def prep_core_inputs(x, w_norm, wq, wk, wv, wo, c):
    j = c % 4
    b = c // 4
    wn = w_norm.astype(np.float32)[:, None]
    xb = x[b].astype(BF16)
    xbT = np.ascontiguousarray(xb.T)
    cols_q = []
    for i in range(4):
        hA, hB = 2 * j + 8 * i, 2 * j + 1 + 8 * i
        cols_q += list(64 * hA + PERM64) + list(64 * hB + PERM64)
    wq_c = np.ascontiguousarray((wn * wq)[:, cols_q]).astype(BF16)
    cols_k = list(64 * (2 * j) + PERM64) + list(64 * (2 * j + 1) + PERM64)
    wk_c = np.ascontiguousarray((wn * wk)[:, cols_k]).astype(BF16)
    cols_v = list(64 * (2 * j) + np.arange(64)) + list(64 * (2 * j + 1) + np.arange(64))
    wv_c = np.ascontiguousarray((wn * wv)[:, cols_v]).astype(BF16)
    rows = 64 * _ag_head_of_row(np.arange(2048)) + (np.arange(2048) % 64)
    wo_c = np.ascontiguousarray(wo[rows][:, 512 * j:512 * (j + 1)]).astype(BF16)
    c128, s128 = _rope_tables(x.shape[1])
    return {"xbT": xbT, "wq": wq_c, "wk": wk_c, "wv": wv_c,
            "wo": wo_c, "c128": c128, "s128": s128, "tri": _tri_mask(),
            "perm": _perm128()}


_NC_CACHE = {}


def kernel(x, w_norm, wq, wk, wv, wo):
    x = np.asarray(x); w_norm = np.asarray(w_norm)
    wq = np.asarray(wq); wk = np.asarray(wk)
    wv = np.asarray(wv); wo = np.asarray(wo)
    if "nc" not in _NC_CACHE:
        _NC_CACHE["nc"] = build_nc(S, D, groups=4, num_devices=8)
    nc = _NC_CACHE["nc"]
    in_maps = [prep_core_inputs(x, w_norm, wq, wk, wv, wo, c) for c in range(NCORE)]
    from concourse.bass_utils import run_bass_kernel_spmd
    res = run_bass_kernel_spmd(nc, in_maps, core_ids=list(range(NCORE)))
    out = np.zeros((B, S, D), np.float32)
    for c in range(NCORE):
        b, j = c // 4, c % 4
        out[b, :, 512 * j:512 * (j + 1)] = res.results[c]["outT"].T
    return out
